# revision 1
# baseline (speedup 1.0000x reference)
"""GCN block (2-layer GCNConv + ReLU) on 8 Trainium2 NeuronCores.

Strategy (1D node partitioning per the sharding hint):
  - Core c owns target nodes [c*N/8, (c+1)*N/8) and every edge whose target
    (col) lands there.
  - Aggregation is reordered before the weight matmul: A_norm @ (x W) ==
    (A_norm @ x) W, so each layer gathers raw table rows, segment-sums them
    into 128-target-node blocks, then applies the dense 96x96 weights.
  - Segment-sum runs on the tensor engine: for each 128-edge chunk of the
    target-sorted edge stream, a selection matrix S[e, m] = norm[e] *
    (localcol[e] == m) is built on the vector engine with one dual-op
    tensor_scalar (is_equal then mult against an iota tile), and
    psum[128 targets, 96] += S.T @ M accumulates over the block's chunks.
    Chunks are packed densely (they may straddle block boundaries; each
    (chunk, block) segment gets its own S/matmul with zeros off-segment).
  - Self-loops of full blocks skip the gather: their table rows are a
    contiguous load and a diagonal S carries dinv^2.
  - Messages M are gathered from a fp16 [N, 96] table in DRAM with
    per-chunk indirect DMAs ([128,1] per-partition offsets - the only
    indirect form this runtime supports).
  - Layer 1 computes T2 = relu((A@x)W1 + b1) @ W2 for owned nodes
    (W2 folded in while the data is feature-major), then an 8-core
    AllGather rebuilds the full table for layer 2's gathers.
  - Layer 2 is aggregation + bias + relu only, written node-major.
"""

import os
import sys

for _p in ("/opt/trn_rl_repo", "/root/.axon_site/_ro/trn_rl_repo"):
    if os.path.isdir(_p) and _p not in sys.path:
        sys.path.insert(0, _p)

import numpy as np

import concourse.bass as bass
import concourse.bacc as bacc
import concourse.mybir as mybir
import concourse.tile as tile
from concourse import bass_utils

F16 = mybir.dt.float16
F32 = mybir.dt.float32
I32 = mybir.dt.int32

P = 128          # partitions / edges per chunk / nodes per target block
D = 96           # feature dim
NCORES = 8


def _preprocess(row, col, ew, N):
    """Bucket edges by owning core, sort by target, pack densely into
    128-edge chunks shared across cores (per-block counts padded to the
    max over cores so one SPMD program fits all eight).

    Returns per-core gather/selection metadata plus the segment schedule
    (chunk, block, first, last) that drives program generation.
    """
    npc = N // NCORES
    nblk = (npc + P - 1) // P
    nfull = npc // P          # blocks whose self-loops use the direct path

    deg = np.bincount(col, weights=ew, minlength=N) + 1.0
    dinv = (1.0 / np.sqrt(deg)).astype(np.float32)
    norm = (dinv[row] * ew * dinv[col]).astype(np.float32)
    selfn = (dinv * dinv).astype(np.float32)

    # per-core edge lists sorted by local target; self-loops only for the
    # partial tail block (full blocks handle them without a gather)
    cores = []
    counts_all = []
    nb = np.zeros(nblk, dtype=np.int64)
    tail = npc - nfull * P
    for c in range(NCORES):
        lo, hi = c * npc, (c + 1) * npc
        m = (col >= lo) & (col < hi)
        r = np.asarray(row[m], dtype=np.int64)
        cl = np.asarray(col[m] - lo, dtype=np.int64)
        w = norm[m]
        if tail:
            tn = np.arange(nfull * P, npc, dtype=np.int64)
            r = np.concatenate([r, tn + lo])
            cl = np.concatenate([cl, tn])
            w = np.concatenate([w, selfn[lo + tn]])
        order = np.argsort(cl, kind="stable")
        r, cl, w = r[order], cl[order], w[order]
        counts = np.bincount(cl // P, minlength=nblk)
        cores.append((r, cl, w))
        counts_all.append(counts)
        nb = np.maximum(nb, counts)

    L = int(nb.sum())
    nchunks = (L + P - 1) // P
    Lpad = nchunks * P
    nb_pad = nb.copy()
    nb_pad[-1] += Lpad - L      # stream tail padding charged to last block

    # block start positions in the padded stream, and the segment schedule
    starts = np.zeros(nblk + 1, dtype=np.int64)
    starts[1:] = np.cumsum(nb_pad)
    segs = []  # (chunk, block, first, last, lane_lo, lane_hi)
    for b in range(nblk):
        s, e = int(starts[b]), int(starts[b + 1])
        c0, c1 = s // P, (e - 1) // P
        for cch in range(c0, c1 + 1):
            lo_ = max(s, cch * P) - cch * P
            hi_ = min(e, (cch + 1) * P) - cch * P
            segs.append((cch, b, cch == c0, cch == c1, lo_, hi_))
    nseg = len(segs)

    rowidx = np.zeros((NCORES, P, nchunks), np.int32)
    colseg = np.zeros((NCORES, P, nseg), np.float32)
    wseg = np.zeros((NCORES, P, nseg), np.float32)
    selfw = np.zeros((NCORES, P, max(nfull, 1)), np.float32)
    for c in range(NCORES):
        r, cl, w = cores[c]
        counts = counts_all[c]
        # build the padded stream for this core
        sr = np.zeros(Lpad, np.int64)
        scl = np.zeros(Lpad, np.int64)
        sw = np.zeros(Lpad, np.float32)
        e0 = 0
        for b in range(nblk):
            n = int(counts[b])
            s = int(starts[b])
            sr[s:s + n] = r[e0:e0 + n]
            scl[s:s + n] = cl[e0:e0 + n] - b * P
            scl[s + n:int(starts[b + 1])] = 0
            sw[s:s + n] = w[e0:e0 + n]
            e0 += n
        rowidx[c] = sr.reshape(nchunks, P).T
        scl2 = scl.reshape(nchunks, P).T
        sw2 = sw.reshape(nchunks, P).T
        for si, (cch, b, _f, _l, lo_, hi_) in enumerate(segs):
            colseg[c, lo_:hi_, si] = scl2[lo_:hi_, cch]
            wseg[c, lo_:hi_, si] = sw2[lo_:hi_, cch]
        lo = c * npc
        for b in range(nfull):
            selfw[c, :, b] = selfn[lo + b * P: lo + (b + 1) * P]

    return (rowidx, colseg, wseg, selfw, segs, nchunks, npc, nblk, nfull)


def _build_program(N, npc, nblk, nfull, nchunks, segs, repeat=1,
                   no_coll=False, mode='full'):
    nseg = len(segs)
    nc = bacc.Bacc("TRN2", target_bir_lowering=False, debug=False,
                   enable_asserts=False, num_devices=NCORES)

    t1 = nc.dram_tensor("t1", [N, D], F16, kind="ExternalInput").ap()
    xo_d = nc.dram_tensor("x_own", [nblk * P, D], F16, kind="ExternalInput").ap()
    rowidx_d = nc.dram_tensor("rowidx", [P, nchunks], I32, kind="ExternalInput").ap()
    colseg_d = nc.dram_tensor("colseg", [P, nseg], F32, kind="ExternalInput").ap()
    wseg_d = nc.dram_tensor("wseg", [P, nseg], F32, kind="ExternalInput").ap()
    selfw_d = nc.dram_tensor("selfw", [P, max(nfull, 1)], F32,
                             kind="ExternalInput").ap()
    iota_d = nc.dram_tensor("iota", [P, P], F16, kind="ExternalInput").ap()
    iotac_d = nc.dram_tensor("iotac", [P, 1], F32, kind="ExternalInput").ap()
    iotaf_d = nc.dram_tensor("iotaf", [P, P], F32, kind="ExternalInput").ap()
    ident_d = nc.dram_tensor("ident", [P, P], F16, kind="ExternalInput").ap()
    w1_d = nc.dram_tensor("w1", [D, D], F16, kind="ExternalInput").ap()
    w2_d = nc.dram_tensor("w2", [D, D], F16, kind="ExternalInput").ap()
    b1_d = nc.dram_tensor("b1", [D, 1], F32, kind="ExternalInput").ap()
    b2rep_d = nc.dram_tensor("b2rep", [P, D], F32, kind="ExternalInput").ap()
    out_d = nc.dram_tensor("out", [nblk * P, D], F32, kind="ExternalOutput").ap()

    with tile.TileContext(nc) as tc:
        with (
            tc.tile_pool(name="const", bufs=1) as const_pool,
            tc.tile_pool(name="meta", bufs=1) as meta_pool,
            tc.tile_pool(name="gath", bufs=6) as g_pool,
            tc.tile_pool(name="smat", bufs=16) as s_pool,
            tc.tile_pool(name="sbig", bufs=4) as sbig_pool,
            tc.tile_pool(name="work", bufs=2) as w_pool,
            tc.tile_pool(name="own", bufs=2) as own_pool,
            tc.tile_pool(name="pagg", bufs=3, space="PSUM") as pagg_pool,
            tc.tile_pool(name="pmisc", bufs=1, space="PSUM") as pmisc_pool,
            tc.tile_pool(name="dram", bufs=1, space="DRAM") as dram_pool,
        ):
            iota_sb = const_pool.tile([P, P], F16, tag="iota")
            iotac_sb = const_pool.tile([P, 1], F32, tag="iotac")
            iotaf_sb = const_pool.tile([P, P], F32, tag="iotaf")
            ident_sb = const_pool.tile([P, P], F16, tag="ident")
            w1_sb = const_pool.tile([D, D], F16, tag="w1")
            w2_sb = const_pool.tile([D, D], F16, tag="w2")
            b1_sb = const_pool.tile([D, 1], F32, tag="b1")
            b2rep_sb = const_pool.tile([P, D], F32, tag="b2rep")
            nc.sync.dma_start(iota_sb[:], iota_d[:])
            nc.sync.dma_start(iotac_sb[:], iotac_d[:])
            nc.sync.dma_start(iotaf_sb[:], iotaf_d[:])
            nc.sync.dma_start(ident_sb[:], ident_d[:])
            nc.sync.dma_start(w1_sb[:], w1_d[:])
            nc.sync.dma_start(w2_sb[:], w2_d[:])
            nc.sync.dma_start(b1_sb[:], b1_d[:])
            nc.sync.dma_start(b2rep_sb[:], b2rep_d[:])

            rowidx_sb = meta_pool.tile([P, nchunks], I32, tag="rowidx")
            colseg_sb = meta_pool.tile([P, nseg], F32, tag="colseg")
            wseg_sb = meta_pool.tile([P, nseg], F32, tag="wseg")
            selfw_sb = meta_pool.tile([P, max(nfull, 1)], F32, tag="selfw")
            nc.sync.dma_start(rowidx_sb[:], rowidx_d[:])
            nc.sync.dma_start(colseg_sb[:], colseg_d[:])
            nc.sync.dma_start(wseg_sb[:], wseg_d[:])
            nc.sync.dma_start(selfw_sb[:], selfw_d[:])

            t2_own = dram_pool.tile([nblk * P, D], F16, tag="t2own")
            t2_fulls = [
                dram_pool.tile([N, D], F16, tag=f"t2full{r}",
                               addr_space="Shared", name=f"t2full{r}")
                for r in range(repeat)
            ]

            def post_block(layer, b, psum_agg):
                rows = min(P, npc - b * P)
                if layer == 0:
                    agg_sb = w_pool.tile([P, P], F16, tag="agg_sb")
                    nc.vector.tensor_copy(agg_sb[:, :D], psum_agg[:])
                    ptr1 = pmisc_pool.tile([P, P], F16, tag="tr1")
                    nc.tensor.transpose(ptr1[:], agg_sb[:], ident_sb[:])
                    aggT_sb = w_pool.tile([D, P], F16, tag="aggT")
                    nc.scalar.activation(
                        aggT_sb[:], ptr1[:D, :],
                        mybir.ActivationFunctionType.Copy)
                    pz = pmisc_pool.tile([D, P], F32, tag="z")
                    nc.tensor.matmul(out=pz[:], lhsT=w1_sb[:], rhs=aggT_sb[:],
                                     start=True, stop=True)
                    h1T_sb = w_pool.tile([P, P], F16, tag="h1T")
                    nc.scalar.activation(
                        h1T_sb[:D, :], pz[:],
                        mybir.ActivationFunctionType.Relu,
                        bias=b1_sb[:], scale=1.0)
                    pt2 = pmisc_pool.tile([D, P], F32, tag="t2")
                    nc.tensor.matmul(out=pt2[:], lhsT=w2_sb[:],
                                     rhs=h1T_sb[:D, :], start=True, stop=True)
                    t2T_sb = w_pool.tile([P, P], F16, tag="t2T")
                    nc.vector.tensor_copy(t2T_sb[:D, :], pt2[:])
                    ptr2 = pmisc_pool.tile([P, P], F16, tag="tr2")
                    nc.tensor.transpose(ptr2[:], t2T_sb[:], ident_sb[:])
                    t2_sb = w_pool.tile([P, D], F16, tag="t2n")
                    nc.vector.tensor_copy(t2_sb[:], ptr2[:, :D])
                    nc.sync.dma_start(
                        t2_own[b * P:b * P + rows, :], t2_sb[:rows, :])
                else:
                    tmp_sb = w_pool.tile([P, D], F32, tag="tmp")
                    nc.vector.tensor_tensor(
                        out=tmp_sb[:], in0=psum_agg[:], in1=b2rep_sb[:],
                        op=mybir.AluOpType.add)
                    o_sb = w_pool.tile([P, D], F32, tag="osb")
                    nc.scalar.activation(
                        o_sb[:], tmp_sb[:],
                        mybir.ActivationFunctionType.Relu)
                    nc.sync.dma_start(out_d[b * P:(b + 1) * P, :], o_sb[:])

            for rep_i, layer in enumerate([0, 1] * repeat):
                t2_full = t2_fulls[rep_i // 2]
                table = t1 if layer == 0 else t2_full[:]
                own_src = xo_d if layer == 0 else t2_own[:]
                psums = {}
                si = 0
                GBC = 16
                for c0 in range(0, nchunks, GBC):
                    gn = min(GBC, nchunks - c0)
                    gbuf = g_pool.tile([P, GBC * D], F16, tag="gbuf",
                                       name="gbuf")
                    for g in range(gn):
                        nc.gpsimd.indirect_dma_start(
                            out=gbuf[:, g * D:(g + 1) * D],
                            out_offset=None,
                            in_=table,
                            in_offset=bass.IndirectOffsetOnAxis(
                                ap=rowidx_sb[:, c0 + g:c0 + g + 1], axis=0),
                        )
                    gbuf2 = g_pool.tile([P, GBC * D], F16, tag="gbuf2",
                                        name="gbuf2")
                    nc.vector.tensor_copy(gbuf2[:, :gn * D], gbuf[:, :gn * D])
                    for cch in range(c0, c0 + gn):
                      while (mode != 'gonly') and si < nseg and segs[si][0] == cch:
                        _c, b, first, last, _lo, _hi = segs[si]
                        if first:
                            psums[b] = pagg_pool.tile([P, D], F32, tag="agg",
                                                      name="pagg")
                            if b < nfull:
                                own_sb = own_pool.tile([P, D], F16, tag="own")
                                nc.sync.dma_start(
                                    own_sb[:], own_src[b * P:(b + 1) * P, :])
                                sdiag = s_pool.tile([P, P], F16, tag="s")
                                nc.vector.tensor_scalar(
                                    out=sdiag[:],
                                    in0=iota_sb[:],
                                    scalar1=iotac_sb[:],
                                    scalar2=selfw_sb[:, b:b + 1],
                                    op0=mybir.AluOpType.is_equal,
                                    op1=mybir.AluOpType.mult,
                                )
                                nc.tensor.matmul(
                                    out=psums[b][:], lhsT=sdiag[:],
                                    rhs=own_sb[:], start=True, stop=False)
                        GBS = 16
                        if si % GBS == 0:
                            gn2 = min(GBS, nseg - si)
                            s_big = sbig_pool.tile(
                                [P, GBS * P], F16, tag="sbig", name="sbig")
                            vw = s_big[:, :gn2 * P].rearrange(
                                "p (g m) -> p g m", m=P)
                            ia = iotaf_sb[:]
                            ca = colseg_sb[:, si:si + gn2]
                            wa = wseg_sb[:, si:si + gn2]
                            ap_i = bass.AP(ia.tensor, ia.offset,
                                           [list(ia.ap[0]), [0, gn2], [1, P]])
                            ap_c = bass.AP(ca.tensor, ca.offset,
                                           [list(ca.ap[0]), list(ca.ap[1]),
                                            [0, P]])
                            ap_w = bass.AP(wa.tensor, wa.offset,
                                           [list(wa.ap[0]), list(wa.ap[1]),
                                            [0, P]])
                            nc.vector.tensor_tensor(
                                out=vw, in0=ap_i, in1=ap_c,
                                op=mybir.AluOpType.is_equal)
                            nc.vector.tensor_tensor(
                                out=vw, in0=vw, in1=ap_w,
                                op=mybir.AluOpType.mult)
                        s_t = s_big[:, (si % GBS) * P:(si % GBS + 1) * P]
                        nc.tensor.matmul(
                            out=psums[b][:],
                            lhsT=s_t,
                            rhs=gbuf2[:, (cch - c0) * D:(cch - c0 + 1) * D],
                            start=(first and b >= nfull),
                            stop=last,
                        )
                        if last:
                            post_block(layer, b, psums.pop(b))
                        si += 1

                if layer == 0 and not no_coll:
                    nc.gpsimd.collective_compute(
                        "AllGather",
                        mybir.AluOpType.bypass,
                        replica_groups=[list(range(NCORES))],
                        ins=[t2_own[:npc, :]],
                        outs=[t2_full[:]],
                    )

    nc.compile()
    return nc


_CACHE = {}


def _get_program(N, npc, nblk, nfull, nchunks, segs, repeat=1,
                 no_coll=False, mode='full'):
    key = (N, npc, nblk, nfull, nchunks, tuple(segs), repeat, no_coll, mode)
    if key not in _CACHE:
        _CACHE[key] = _build_program(N, npc, nblk, nfull, nchunks, segs,
                                     repeat=repeat, no_coll=no_coll, mode=mode)
    return _CACHE[key]


def _make_inputs(x, W1, b1, W2, b2, pre):
    rowidx, colseg, wseg, selfw, segs, nchunks, npc, nblk, nfull = pre
    t1 = np.asarray(x, np.float32).astype(np.float16)
    common = {
        "t1": t1,
        "iota": np.tile(np.arange(P, dtype=np.float16), (P, 1)),
        "iotac": np.arange(P, dtype=np.float32).reshape(P, 1),
        "iotaf": np.tile(np.arange(P, dtype=np.float32), (P, 1)),
        "ident": np.eye(P, dtype=np.float16),
        "w1": np.asarray(W1, np.float32).astype(np.float16),
        "w2": np.asarray(W2, np.float32).astype(np.float16),
        "b1": np.asarray(b1, np.float32).reshape(D, 1),
        "b2rep": np.tile(np.asarray(b2, np.float32).reshape(1, D), (P, 1)),
    }
    in_maps = []
    for c in range(NCORES):
        xo = np.zeros((nblk * P, D), np.float16)
        xo[:npc] = t1[c * npc:(c + 1) * npc]
        m = dict(common)
        m["x_own"] = xo
        m["rowidx"] = rowidx[c]
        m["colseg"] = colseg[c]
        m["wseg"] = wseg[c]
        m["selfw"] = selfw[c]
        in_maps.append(m)
    return in_maps


def kernel(x, edge_index, edge_weight, batch, W1, b1, W2, b2, **_unused):
    x = np.asarray(x, dtype=np.float32)
    edge_index = np.asarray(edge_index)
    ew = np.asarray(edge_weight, dtype=np.float32)
    N = x.shape[0]
    row = np.asarray(edge_index[0], dtype=np.int64)
    col = np.asarray(edge_index[1], dtype=np.int64)

    pre = _preprocess(row, col, ew, N)
    rowidx, colseg, wseg, selfw, segs, nchunks, npc, nblk, nfull = pre
    nc = _get_program(N, npc, nblk, nfull, nchunks, segs)
    in_maps = _make_inputs(x, W1, b1, W2, b2, pre)

    res = bass_utils.run_bass_kernel_spmd(nc, in_maps, core_ids=list(range(NCORES)))
    out = np.concatenate([res.results[c]["out"][:npc] for c in range(NCORES)],
                         axis=0)
    return out.astype(np.float32)



# revision 7
# speedup vs baseline: 270.0018x; 270.0018x over previous
"""GCN block (2-layer GCNConv + ReLU) on 8 Trainium2 NeuronCores — v2.

Same 1D target partitioning as v1, but the per-edge table gathers go
through InstDMAGatherAnt (up to 1024 rows per instruction) instead of one
indirect DMA per 128-edge chunk. dma_gather takes int16 indices, so each
target block's edge run is split into a low-source half (row < 25000) and
a high-source half, each padded to a 128-slot chunk boundary; a batch of
up to 8 consecutive same-half chunks becomes one gather against the
corresponding half of the 128-column-padded table. Indices are packed
[16, n/16] wrapped and replicated across the 8 GPSIMD Q7 banks; gather
completion is signaled on rotating semaphores (the tile tracker does not
model the async DMA landing).

Layer tails, self-loop handling, S-matrix segment matmuls and the
inter-layer AllGather are unchanged from v1 (segments now coincide with
chunks because runs are chunk-aligned).
"""

import os
import sys

for _p in ("/opt/trn_rl_repo", "/root/.axon_site/_ro/trn_rl_repo"):
    if os.path.isdir(_p) and _p not in sys.path:
        sys.path.insert(0, _p)

import numpy as np

import concourse.bass as bass
import concourse.bacc as bacc
import concourse.mybir as mybir
import concourse.tile as tile
from concourse import bass_utils

F16 = mybir.dt.float16
F32 = mybir.dt.float32
I16 = mybir.dt.int16

P = 128
D = 96
ES = 128          # padded table row (256B, dma_gather elem_size)
NCORES = 8
HALF = 25000      # low-source rows [0, HALF), high [HALF, N)
MAXCH = 8         # chunks per gather batch (num_idxs <= 1024)
NSEM = 6


def _preprocess(row, col, ew, N):
    """Per-core edge streams: per target block, [low-half run | high-half
    run], each padded to a 128-slot chunk boundary shared across cores.

    Returns (rowidx16, colseg, wseg, selfw, batches, segs, nchunks, npc,
    nblk, nfull). rowidx16 is the [NCORES, 128, idxcols] replicated int16
    index table; batches is [(chunk0, nch, half, idxcol0)].
    """
    npc = N // NCORES
    nblk = (npc + P - 1) // P
    nfull = npc // P
    tail = npc - nfull * P

    deg = np.bincount(col, weights=ew, minlength=N) + 1.0
    dinv = (1.0 / np.sqrt(deg)).astype(np.float32)
    norm = (dinv[row] * ew * dinv[col]).astype(np.float32)
    selfn = (dinv * dinv).astype(np.float32)

    cores = []
    nchL = np.zeros(nblk, dtype=np.int64)   # chunks per (block, half)
    nchH = np.zeros(nblk, dtype=np.int64)
    for c in range(NCORES):
        lo, hi = c * npc, (c + 1) * npc
        m = (col >= lo) & (col < hi)
        r = np.asarray(row[m], dtype=np.int64)
        cl = np.asarray(col[m] - lo, dtype=np.int64)
        w = norm[m]
        if tail:
            tn = np.arange(nfull * P, npc, dtype=np.int64)
            r = np.concatenate([r, tn + lo])
            cl = np.concatenate([cl, tn])
            w = np.concatenate([w, selfn[lo + tn]])
        half = (r >= HALF).astype(np.int64)
        order = np.lexsort((half, cl // P))   # by block, then half
        r, cl, w, half = r[order], cl[order], w[order], half[order]
        blk = cl // P
        cores.append((r, cl, w, half, blk))
        for b in range(nblk):
            mb = blk == b
            nL = int((half[mb] == 0).sum())
            nH = int((half[mb] == 1).sum())
            nchL[b] = max(nchL[b], (nL + P - 1) // P)
            nchH[b] = max(nchH[b], (nH + P - 1) // P)
        # every (block, half) needs at least one chunk so the schedule is
        # uniform; empty halves gather dummy rows with w=0
    nchL = np.maximum(nchL, 1)
    nchH = np.maximum(nchH, 1)

    # chunk schedule: per block, two half-runs; alternate the half order
    # per block so consecutive same-half runs merge into longer gather
    # batches across the block boundary
    segs = []      # (chunk, block, first, last)
    runs = []      # (chunk0, nch, half) merged same-half runs
    halves_of = []  # per block, ordered halves
    cch = 0
    for b in range(nblk):
        order = ((0, 1) if b % 2 == 0 else (1, 0))
        pair = []
        for pos, hf in enumerate(order):
            nch = int((nchL if hf == 0 else nchH)[b])
            pair.append((hf, nch))
            for k in range(nch):
                segs.append((cch + k, b, pos == 0 and k == 0,
                             pos == 1 and k == nch - 1))
            if runs and runs[-1][2] == hf and runs[-1][0] + runs[-1][1] == cch:
                runs[-1] = (runs[-1][0], runs[-1][1] + nch, hf)
            else:
                runs.append((cch, nch, hf))
            cch += nch
        halves_of.append(pair)
    nchunks = cch

    # split runs into batches of <= MAXCH chunks, sized evenly
    batches = []   # (chunk0, nch, half, idxcol0)
    idxcol = 0
    for (c0, nch, hf) in runs:
        nparts = (nch + MAXCH - 1) // MAXCH
        base, rem = divmod(nch, nparts)
        ofs = c0
        for pi in range(nparts):
            n = base + (1 if pi < rem else 0)
            batches.append((ofs, n, hf, idxcol))
            idxcol += n * P // 16
            ofs += n
    idxcols = idxcol
    L = nchunks * P

    rowidx16 = np.zeros((NCORES, P, idxcols), np.int16)
    colseg = np.zeros((NCORES, P, nchunks), np.float32)
    wseg = np.zeros((NCORES, P, nchunks), np.float32)
    selfw = np.zeros((NCORES, P, max(nfull, 1)), np.float32)
    for c in range(NCORES):
        r, cl, w, half, blk = cores[c]
        sr = np.zeros(L, np.int64)
        scl = np.zeros(L, np.int64)
        sw = np.zeros(L, np.float32)
        shf = np.zeros(L, np.int64)
        pos = 0
        e0 = 0
        cpos = 0
        for b in range(nblk):
            mb = blk == b
            nb_tot = int(mb.sum())
            eb_r = r[e0:e0 + nb_tot]
            eb_cl = cl[e0:e0 + nb_tot]
            eb_w = w[e0:e0 + nb_tot]
            eb_h = half[e0:e0 + nb_tot]
            e0 += nb_tot
            for hf, nch in halves_of[b]:
                mh = eb_h == (hf)
                n = int(mh.sum())
                span = nch * P
                sl = slice(cpos, cpos + span)
                sr[sl.start:sl.start + n] = eb_r[mh]
                scl[sl.start:sl.start + n] = eb_cl[mh] - b * P
                sw[sl.start:sl.start + n] = eb_w[mh]
                # pad slots: dummy row inside this half, weight 0
                sr[sl.start + n:sl.stop] = HALF * hf
                shf[sl] = hf
                cpos += span
        assert cpos == L
        rel = sr - HALF * shf
        assert rel.min() >= 0 and rel.max() < 32768
        scl2 = scl.reshape(nchunks, P)
        sw2 = sw.reshape(nchunks, P)
        colseg[c] = scl2.T
        wseg[c] = sw2.T
        rel2 = rel.reshape(nchunks, P)
        for (c0, nch, hf, col0) in batches:
            flat = rel2[c0:c0 + nch].reshape(-1).astype(np.int16)
            ncols = nch * P // 16
            rowidx16[c, :16, col0:col0 + ncols] = (
                flat.reshape(ncols, 16).T)
        for n in range(1, 8):
            rowidx16[c, 16 * n:16 * (n + 1)] = rowidx16[c, :16]
        lo = c * npc
        for b in range(nfull):
            selfw[c, :, b] = selfn[lo + b * P: lo + (b + 1) * P]

    return (rowidx16, colseg, wseg, selfw, batches, segs, nchunks, npc,
            nblk, nfull)


def _build_program(N, npc, nblk, nfull, nchunks, idxcols, batches, segs,
                   repeat=1, no_coll=False, mode='full', loop_reps=0):
    nseg = len(segs)
    nc = bacc.Bacc("TRN2", target_bir_lowering=False, debug=False,
                   enable_asserts=False, num_devices=NCORES,
                   dynamic_dma_scratch_size=131072)

    t1 = nc.dram_tensor("t1", [N, ES], F16, kind="ExternalInput").ap()
    xo_d = nc.dram_tensor("x_own", [nblk * P, D], F16,
                          kind="ExternalInput").ap()
    rowidx_d = nc.dram_tensor("rowidx", [P, idxcols], I16,
                              kind="ExternalInput").ap()
    colseg_d = nc.dram_tensor("colseg", [P, nseg], F32,
                              kind="ExternalInput").ap()
    wseg_d = nc.dram_tensor("wseg", [P, nseg], F32, kind="ExternalInput").ap()
    selfw_d = nc.dram_tensor("selfw", [P, max(nfull, 1)], F32,
                             kind="ExternalInput").ap()
    iota_d = nc.dram_tensor("iota", [P, P], F16, kind="ExternalInput").ap()
    iotac_d = nc.dram_tensor("iotac", [P, 1], F32, kind="ExternalInput").ap()
    iotaf_d = nc.dram_tensor("iotaf", [P, P], F32, kind="ExternalInput").ap()
    ident_d = nc.dram_tensor("ident", [P, P], F16, kind="ExternalInput").ap()
    w1_d = nc.dram_tensor("w1", [D, D], F16, kind="ExternalInput").ap()
    w2_d = nc.dram_tensor("w2", [D, D], F16, kind="ExternalInput").ap()
    b1_d = nc.dram_tensor("b1", [D, 1], F32, kind="ExternalInput").ap()
    b2rep_d = nc.dram_tensor("b2rep", [P, D], F32, kind="ExternalInput").ap()
    out_d = nc.dram_tensor("out", [nblk * P, D], F32,
                           kind="ExternalOutput").ap()

    with tile.TileContext(nc) as tc:
        with (
            tc.tile_pool(name="const", bufs=1) as const_pool,
            tc.tile_pool(name="meta", bufs=1) as meta_pool,
            tc.tile_pool(name="gath", bufs=6) as g_pool,
            tc.tile_pool(name="smat", bufs=16) as s_pool,
            tc.tile_pool(name="sbig", bufs=4) as sbig_pool,
            tc.tile_pool(name="work", bufs=2) as w_pool,
            tc.tile_pool(name="own", bufs=2) as own_pool,
            tc.tile_pool(name="pagg", bufs=3, space="PSUM") as pagg_pool,
            tc.tile_pool(name="pmisc", bufs=1, space="PSUM") as pmisc_pool,
            tc.tile_pool(name="dram", bufs=1, space="DRAM") as dram_pool,
        ):
            iota_sb = const_pool.tile([P, P], F16, tag="iota")
            iotac_sb = const_pool.tile([P, 1], F32, tag="iotac")
            iotaf_sb = const_pool.tile([P, P], F32, tag="iotaf")
            ident_sb = const_pool.tile([P, P], F16, tag="ident")
            w1_sb = const_pool.tile([D, D], F16, tag="w1")
            w2_sb = const_pool.tile([D, D], F16, tag="w2")
            b1_sb = const_pool.tile([D, 1], F32, tag="b1")
            b2rep_sb = const_pool.tile([P, D], F32, tag="b2rep")
            nc.sync.dma_start(iota_sb[:], iota_d[:])
            nc.sync.dma_start(iotac_sb[:], iotac_d[:])
            nc.sync.dma_start(iotaf_sb[:], iotaf_d[:])
            nc.sync.dma_start(ident_sb[:], ident_d[:])
            nc.sync.dma_start(w1_sb[:], w1_d[:])
            nc.sync.dma_start(w2_sb[:], w2_d[:])
            nc.sync.dma_start(b1_sb[:], b1_d[:])
            nc.sync.dma_start(b2rep_sb[:], b2rep_d[:])

            rowidx_sb = meta_pool.tile([P, idxcols], I16, tag="rowidx")
            colseg_sb = meta_pool.tile([P, nseg], F32, tag="colseg")
            wseg_sb = meta_pool.tile([P, nseg], F32, tag="wseg")
            selfw_sb = meta_pool.tile([P, max(nfull, 1)], F32, tag="selfw")
            nc.sync.dma_start(rowidx_sb[:], rowidx_d[:])
            nc.sync.dma_start(colseg_sb[:], colseg_d[:])
            nc.sync.dma_start(wseg_sb[:], wseg_d[:])
            nc.sync.dma_start(selfw_sb[:], selfw_d[:])

            t2_own = dram_pool.tile([nblk * P, ES], F16, tag="t2own")
            t2_fulls = [
                dram_pool.tile([N, ES], F16, tag=f"t2full{r}",
                               addr_space="Shared", name=f"t2full{r}")
                for r in range(repeat if loop_reps == 0 else 1)
            ]
            sems = [nc.alloc_semaphore(f"gd{i}") for i in range(NSEM)]

            zpad_sb = const_pool.tile([P, ES - D], F16, tag="zpad")
            nc.vector.memset(zpad_sb[:], 0.0)
            for b in range(nblk):
                nc.sync.dma_start(t2_own[b * P:(b + 1) * P, D:ES],
                                  zpad_sb[:])

            def post_block(layer, b, psum_agg):
                rows = min(P, npc - b * P)
                if layer == 0:
                    agg_sb = w_pool.tile([P, P], F16, tag="agg_sb")
                    nc.vector.tensor_copy(agg_sb[:, :D], psum_agg[:])
                    ptr1 = pmisc_pool.tile([P, P], F16, tag="tr1")
                    nc.tensor.transpose(ptr1[:], agg_sb[:], ident_sb[:])
                    aggT_sb = w_pool.tile([D, P], F16, tag="aggT")
                    nc.scalar.activation(
                        aggT_sb[:], ptr1[:D, :],
                        mybir.ActivationFunctionType.Copy)
                    pz = pmisc_pool.tile([D, P], F32, tag="z")
                    nc.tensor.matmul(out=pz[:], lhsT=w1_sb[:], rhs=aggT_sb[:],
                                     start=True, stop=True)
                    h1T_sb = w_pool.tile([P, P], F16, tag="h1T")
                    nc.scalar.activation(
                        h1T_sb[:D, :], pz[:],
                        mybir.ActivationFunctionType.Relu,
                        bias=b1_sb[:], scale=1.0)
                    pt2 = pmisc_pool.tile([D, P], F32, tag="t2")
                    nc.tensor.matmul(out=pt2[:], lhsT=w2_sb[:],
                                     rhs=h1T_sb[:D, :], start=True, stop=True)
                    t2T_sb = w_pool.tile([P, P], F16, tag="t2T")
                    nc.vector.tensor_copy(t2T_sb[:D, :], pt2[:])
                    ptr2 = pmisc_pool.tile([P, P], F16, tag="tr2")
                    nc.tensor.transpose(ptr2[:], t2T_sb[:], ident_sb[:])
                    t2_sb = w_pool.tile([P, D], F16, tag="t2n")
                    nc.vector.tensor_copy(t2_sb[:], ptr2[:, :D])
                    nc.sync.dma_start(
                        t2_own[b * P:b * P + rows, 0:D], t2_sb[:rows, :])
                else:
                    tmp_sb = w_pool.tile([P, D], F32, tag="tmp")
                    nc.vector.tensor_tensor(
                        out=tmp_sb[:], in0=psum_agg[:], in1=b2rep_sb[:],
                        op=mybir.AluOpType.add)
                    o_sb = w_pool.tile([P, D], F32, tag="osb")
                    nc.scalar.activation(
                        o_sb[:], tmp_sb[:],
                        mybir.ActivationFunctionType.Relu)
                    nc.sync.dma_start(out_d[b * P:(b + 1) * P, :], o_sb[:])

            def pair_body(t2_full):
              for layer in (0, 1):
                table = t1 if layer == 0 else t2_full[:]
                own_src = xo_d if layer == 0 else t2_own[:, 0:D]
                psums = {}
                si = 0
                for lbi, (c0, nch, hf, col0) in enumerate(batches):
                    bi = layer * len(batches) + lbi
                    gbuf = g_pool.tile([P, MAXCH * ES], F16, tag="gbuf",
                                       name="gbuf")
                    nidx = nch * P
                    tbl = table[0:HALF, :] if hf == 0 else table[HALF:N, :]
                    nc.gpsimd.dma_gather(
                        out_ap=gbuf[:, :nch * ES].rearrange(
                            "p (j e) -> p j e", e=ES),
                        in_ap=tbl,
                        idxs_ap=rowidx_sb[:, col0:col0 + nidx // 16],
                        num_idxs=nidx,
                        num_idxs_reg=nidx,
                        elem_size=ES,
                    ).then_inc(sems[bi % NSEM], 16)
                    if mode == 'gonly':
                        nc.tensor.wait_ge(sems[bi % NSEM],
                                          16 * (bi // NSEM + 1))
                        continue
                    nc.tensor.wait_ge(sems[bi % NSEM], 16 * (bi // NSEM + 1))
                    for j in range(nch):
                        cch = c0 + j
                        _c, b, first, last = segs[si]
                        assert _c == cch
                        if first:
                            psums[b] = pagg_pool.tile([P, D], F32, tag="agg",
                                                      name="pagg")
                            if b < nfull:
                                own_sb = own_pool.tile([P, D], F16, tag="own")
                                nc.sync.dma_start(
                                    own_sb[:], own_src[b * P:(b + 1) * P, :])
                                sdiag = s_pool.tile([P, P], F16, tag="s")
                                nc.vector.tensor_scalar(
                                    out=sdiag[:],
                                    in0=iota_sb[:],
                                    scalar1=iotac_sb[:],
                                    scalar2=selfw_sb[:, b:b + 1],
                                    op0=mybir.AluOpType.is_equal,
                                    op1=mybir.AluOpType.mult,
                                )
                                nc.tensor.matmul(
                                    out=psums[b][:], lhsT=sdiag[:],
                                    rhs=own_sb[:], start=True, stop=False)
                        GBS = 16
                        if si % GBS == 0:
                            gn2 = min(GBS, nseg - si)
                            s_big = sbig_pool.tile(
                                [P, GBS * P], F16, tag="sbig", name="sbig")
                            vw = s_big[:, :gn2 * P].rearrange(
                                "p (g m) -> p g m", m=P)
                            ia = iotaf_sb[:]
                            ca = colseg_sb[:, si:si + gn2]
                            wa = wseg_sb[:, si:si + gn2]
                            ap_i = bass.AP(ia.tensor, ia.offset,
                                           [list(ia.ap[0]), [0, gn2], [1, P]])
                            ap_c = bass.AP(ca.tensor, ca.offset,
                                           [list(ca.ap[0]), list(ca.ap[1]),
                                            [0, P]])
                            ap_w = bass.AP(wa.tensor, wa.offset,
                                           [list(wa.ap[0]), list(wa.ap[1]),
                                            [0, P]])
                            nc.vector.tensor_tensor(
                                out=vw, in0=ap_i, in1=ap_c,
                                op=mybir.AluOpType.is_equal)
                            nc.vector.tensor_tensor(
                                out=vw, in0=vw, in1=ap_w,
                                op=mybir.AluOpType.mult)
                        s_t = s_big[:, (si % GBS) * P:(si % GBS + 1) * P]
                        nc.tensor.matmul(
                            out=psums[b][:],
                            lhsT=s_t,
                            rhs=gbuf[:, j * ES:j * ES + D],
                            start=(first and b >= nfull),
                            stop=last,
                        )
                        if last:
                            post_block(layer, b, psums.pop(b))
                        si += 1

                if layer == 0 and not no_coll:
                    nc.gpsimd.collective_compute(
                        "AllGather",
                        mybir.AluOpType.bypass,
                        replica_groups=[list(range(NCORES))],
                        ins=[t2_own[:npc, :]],
                        outs=[t2_full[:]],
                    )

            if loop_reps > 0:
                with tc.For_i(0, loop_reps):
                    pair_body(t2_fulls[0])
            else:
                for rep_i in range(repeat):
                    pair_body(t2_fulls[rep_i])

    nc.compile()
    return nc


_CACHE = {}


def _get_program(N, npc, nblk, nfull, nchunks, idxcols, batches, segs,
                 repeat=1, no_coll=False, mode='full', loop_reps=0):
    key = (N, npc, nblk, nfull, nchunks, idxcols, tuple(batches), tuple(segs),
           repeat, no_coll, mode, loop_reps)
    if key not in _CACHE:
        _CACHE[key] = _build_program(N, npc, nblk, nfull, nchunks, idxcols,
                                     batches, segs, repeat=repeat,
                                     no_coll=no_coll, mode=mode,
                                     loop_reps=loop_reps)
    return _CACHE[key]


def _make_inputs(x, W1, b1, W2, b2, pre):
    (rowidx16, colseg, wseg, selfw, batches, segs, nchunks, npc, nblk,
     nfull) = pre
    N = x.shape[0]
    t1 = np.zeros((N, ES), np.float16)
    t1[:, :D] = np.asarray(x, np.float32).astype(np.float16)
    common = {
        "t1": t1,
        "iota": np.tile(np.arange(P, dtype=np.float16), (P, 1)),
        "iotac": np.arange(P, dtype=np.float32).reshape(P, 1),
        "iotaf": np.tile(np.arange(P, dtype=np.float32), (P, 1)),
        "ident": np.eye(P, dtype=np.float16),
        "w1": np.asarray(W1, np.float32).astype(np.float16),
        "w2": np.asarray(W2, np.float32).astype(np.float16),
        "b1": np.asarray(b1, np.float32).reshape(D, 1),
        "b2rep": np.tile(np.asarray(b2, np.float32).reshape(1, D), (P, 1)),
    }
    in_maps = []
    for c in range(NCORES):
        xo = np.zeros((nblk * P, D), np.float16)
        xo[:npc] = t1[c * npc:(c + 1) * npc, :D]
        m = dict(common)
        m["x_own"] = xo
        m["rowidx"] = rowidx16[c]
        m["colseg"] = colseg[c]
        m["wseg"] = wseg[c]
        m["selfw"] = selfw[c]
        in_maps.append(m)
    return in_maps


def kernel(x, edge_index, edge_weight, batch, W1, b1, W2, b2, **_unused):
    x = np.asarray(x, dtype=np.float32)
    edge_index = np.asarray(edge_index)
    ew = np.asarray(edge_weight, dtype=np.float32)
    N = x.shape[0]
    row = np.asarray(edge_index[0], dtype=np.int64)
    col = np.asarray(edge_index[1], dtype=np.int64)

    pre = _preprocess(row, col, ew, N)
    (rowidx16, colseg, wseg, selfw, batches, segs, nchunks, npc, nblk,
     nfull) = pre
    idxcols = rowidx16.shape[2]
    nc = _get_program(N, npc, nblk, nfull, nchunks, idxcols, batches, segs)
    in_maps = _make_inputs(x, W1, b1, W2, b2, pre)

    res = bass_utils.run_bass_kernel_spmd(nc, in_maps,
                                          core_ids=list(range(NCORES)))
    out = np.concatenate([res.results[c]["out"][:npc]
                          for c in range(NCORES)], axis=0)
    return out.astype(np.float32)
